# revision 1
# baseline (speedup 1.0000x reference)
"""Lorentz MLA attention kernel for Trainium2, sharded over 8 NeuronCores.

Sharding: tensor-parallel over the 16 attention heads (2 heads per core);
the kv_lora latent projection (wkv_a + RMS norm) is replicated. The output
projection wo is row-parallel: each core produces a partial (2048, 2047)
output; the host sums the 8 partials and applies the final Lorentz lift.

Device-side layout notes:
- Everything flows transposed ([feature, s]) so all matmuls contract on the
  partition axis without transposing x on device (host passes x^T).
- The 193-dim Lorentz q/k contraction is chunked [128 nope] + [64 rope + 1
  time]; the time rows sit at partition 64 of the 65-row "B" tiles (32-aligned,
  as required by the engines' partition-access rule).
- Rope dims are permuted even-pairs-first on the host so rotary is four
  aligned [32, S] ops; weight columns are permuted to match.
- The softmax max-pass is skipped (scores are bounded) and the softmax
  denominator is dropped entirely: it cancels inside the Lorentz centroid
  normalization. V' carries its time coordinate in column 127 (wo rows are
  permuted on the host to match).
"""

import os
import sys
import types

import numpy as np
import ml_dtypes


def _ensure_axon_hooks():
    """Recreate the missing antenv.axon_hooks module so NTFF tracing works."""
    if "antenv.axon_hooks" in sys.modules:
        return
    try:
        import antenv
        from trn_agent_boot.trn_boot import _ntff_profile_via_ctypes

        hook = _ntff_profile_via_ctypes("/opt/axon/libaxon_pjrt.so")
        mod = types.ModuleType("antenv.axon_hooks")
        mod.get_axon_ntff_profile_hook = lambda: hook
        mod.set_axon_ntff_profile_hook = lambda h: None
        sys.modules["antenv.axon_hooks"] = mod
        antenv.axon_hooks = mod
    except Exception:
        pass


_ensure_axon_hooks()

import concourse.bacc as bacc
import concourse.bass as bass
import concourse.tile as tile
from concourse import mybir
import concourse.bass_utils as bass_utils
from concourse.bass_utils import run_bass_kernel_spmd
from concourse.masks import make_identity, make_upper_triangular

# zero-egress container: make the S3 artifact upload in the profile path a no-op
bass_utils.upload_artifacts = lambda tmpdir: tmpdir

F32 = mybir.dt.float32
BF16 = mybir.dt.bfloat16
F32R = mybir.dt.float32r
AF = mybir.ActivationFunctionType
AX = mybir.AxisListType

N_CORES = 8
P = 128
S = 2048          # sequence length
DIM = 2048        # model dim
NDC = DIM // P    # 16 contraction chunks over DIM
NQT = S // P      # 16 q/k tiles of 128
HPC = 2           # heads per core
NOPE = 128
RSP = 64          # rotary space dim
VSP = 127         # v space dim
KV_RANK = 512
EPS_RMS = 1e-6
QH = NOPE + RSP               # 192 q space rows per head
WQ_COLS = HPC * QH            # 384
WB_COLS = HPC * (NOPE + VSP)  # 510
WO_ROWS = HPC * P             # 256
OUT_COLS = DIM - 1            # 2047


def _build_program(exp_scale: float, causal: bool):
    nc = bacc.Bacc("TRN2", target_bir_lowering=False, debug=False,
                   num_devices=N_CORES)

    xT_d = nc.dram_tensor("xT", [DIM, S], BF16, kind="ExternalInput")
    wq_d = nc.dram_tensor("wq", [DIM, WQ_COLS], BF16, kind="ExternalInput")
    wkva_d = nc.dram_tensor("wkva", [DIM, KV_RANK + RSP], BF16, kind="ExternalInput")
    wnormT_d = nc.dram_tensor("wnormT", [P, 4], F32, kind="ExternalInput")
    wkvb_d = nc.dram_tensor("wkvb", [KV_RANK + 1, WB_COLS], BF16, kind="ExternalInput")
    wo_d = nc.dram_tensor("wo", [WO_ROWS, OUT_COLS], BF16, kind="ExternalInput")
    cosT_d = nc.dram_tensor("cosT", [RSP, S], F32, kind="ExternalInput")
    sinT_d = nc.dram_tensor("sinT", [RSP, S], F32, kind="ExternalInput")
    out_d = nc.dram_tensor("out", [S, OUT_COLS], F32, kind="ExternalOutput")
    SL = S // N_CORES
    GR = KV_RANK + RSP + 1  # gathered rows: kvn + kpe + t_row
    xsl_d = nc.dram_tensor("xsl", [DIM, SL], BF16, kind="ExternalInput")
    cossl_d = nc.dram_tensor("cossl", [RSP, SL], F32, kind="ExternalInput")
    sinsl_d = nc.dram_tensor("sinsl", [RSP, SL], F32, kind="ExternalInput")
    gin = nc.dram_tensor("gin", [GR, SL], BF16)
    gout = nc.dram_tensor("gout", [N_CORES, GR, SL], BF16, addr_space="Shared")

    with tile.TileContext(nc) as tc:
        const = tc.alloc_tile_pool(name="const", bufs=1)
        identity = const.tile([P, P], F32)
        make_identity(nc, identity)
        diagmask = const.tile([P, P], F32)
        make_upper_triangular(nc, diagmask, val=1.0, diag=True)
        wnormT = const.tile([P, 4], F32)
        nc.sync.dma_start(out=wnormT[:], in_=wnormT_d[:])
        Lt = const.tile([P, 4, 2], F32)  # [ones | wnorm^2] per latent chunk
        for c in range(4):
            nc.vector.memset(Lt[:, c, 0:1], 1.0)
            nc.vector.tensor_mul(Lt[:, c, 1:2], wnormT[:, c:c + 1], wnormT[:, c:c + 1])
        ones_col = const.tile([P, 1], F32)
        nc.vector.memset(ones_col[:], 1.0)
        eps_b = const.tile([P, 1], F32)
        nc.vector.memset(eps_b[:], EPS_RMS)
        ones_row = const.tile([1, P], F32)
        nc.vector.memset(ones_row[:], 1.0)

        # One long-lived pool; overlapping lifetimes share slots via tags.
        big = tc.alloc_tile_pool(name="big", bufs=1)
        qsA = [big.tile([P, S], F32R, name=f"qsA_{h}", tag=f"qsA_{h}") for h in range(HPC)]
        qsB = [big.tile([RSP + 1, S], F32R, name=f"qsB_{h}", tag=f"qsB_{h}") for h in range(HPC)]
        kv = [big.tile([P, S], BF16, name=f"kv_{c}", tag=f"kv_{c}") for c in range(4)]
        kpe = big.tile([RSP, S], BF16, name="kpe", tag="kpe")
        ksB = [big.tile([RSP + 1, S], F32R, name=f"ksB_{h}", tag=f"ksB_{h}") for h in range(HPC)]
        Vp = [big.tile([P, NQT, P], F32R, name=f"Vp_{h}", tag=f"Vp_{h}") for h in range(HPC)]
        t_row_bf = big.tile([1, S], BF16, name="t_row_bf", tag="t_row_bf")

        # ------------- Phase A: q / kv / k_pe projection (bf16, one pass) ----
        # weight col layout (host): [qnope0|qnope1|qrope0+1 packed|kv|kpe]
        NCH = 512
        NA = S // NCH
        p_wA = tc.alloc_tile_pool(name="p_wA", bufs=1)
        p_xs = tc.alloc_tile_pool(name="p_xs", bufs=1)
        p_psA = tc.alloc_tile_pool(name="p_psA", bufs=4, space="PSUM")
        wKV = []
        for dc in range(NDC):
            w = p_wA.tile([P, KV_RANK + RSP], BF16, name=f"wKV_{dc}", tag=f"wKV_{dc}")
            nc.sync.dma_start(out=w[:], in_=wkva_d[dc * P:(dc + 1) * P, :])
            wKV.append(w)
        wQ = []
        for dc in range(NDC):
            w = p_wA.tile([P, WQ_COLS], BF16, name=f"wQ_{dc}", tag=f"wQ_{dc}")
            nc.sync.dma_start(out=w[:], in_=wq_d[dc * P:(dc + 1) * P, :])
            wQ.append(w)

        def _w_qrope(ps, n0, nn):
            nc.scalar.copy(qsB[0][0:RSP, n0:n0 + nn], ps[0:RSP, :nn])
            nc.scalar.copy(qsB[1][0:RSP, n0:n0 + nn], ps[RSP:P, :nn])

        # --- kv latent: this core's s-slice only, then AllGather -------------
        p_sl = tc.alloc_tile_pool(name="p_sl", bufs=1)
        p_pssl = tc.alloc_tile_pool(name="p_pssl", bufs=2, space="PSUM")
        xsl_t = p_sl.tile([P, NDC, SL], BF16, name="xsl_t", tag="xsl_t")
        for dc in range(NDC):
            nc.sync.dma_start(out=xsl_t[:, dc, :],
                              in_=xsl_d[dc * P:(dc + 1) * P, :])
        cossl = p_sl.tile([RSP, SL], F32, name="cossl", tag="cossl")
        sinsl = p_sl.tile([RSP, SL], F32, name="sinsl", tag="sinsl")
        nc.sync.dma_start(out=cossl[:], in_=cossl_d[:])
        nc.sync.dma_start(out=sinsl[:], in_=sinsl_d[:])

        kvsl = [p_sl.tile([P, SL], F32, name=f"kvsl_{c}", tag=f"kvsl_{c}")
                for c in range(4)]
        kpesl = p_sl.tile([RSP, SL], F32, name="kpesl", tag="kpesl")
        for c in range(4):
            ps = p_pssl.tile([P, SL], F32, name="psl", tag="psl", bufs=2)
            for dc in range(NDC):
                nc.tensor.matmul(ps[:], wKV[dc][:, c * P:(c + 1) * P],
                                 xsl_t[:, dc, :], start=(dc == 0), stop=(dc == NDC - 1))
            nc.vector.tensor_copy(kvsl[c][:], ps[:])
        ps = p_pssl.tile([P, SL], F32, name="psl", tag="psl", bufs=2)
        for dc in range(NDC):
            nc.tensor.matmul(ps[:RSP, :], wKV[dc][:, KV_RANK:],
                             xsl_t[:, dc, :], start=(dc == 0), stop=(dc == NDC - 1))
        nc.vector.tensor_copy(kpesl[:], ps[:RSP, :])

        # RMS stats on the slice
        ps_s = p_pssl.tile([1, SL], F32, name="ps_s", tag="ps_s", bufs=1)
        ps_w = p_pssl.tile([1, SL], F32, name="ps_w", tag="ps_w", bufs=1)
        for c in range(4):
            ksq = p_sl.tile([P, SL], F32, name="ksq", tag="ksq", bufs=2)
            nc.scalar.square(ksq[:], kvsl[c][:])
            nc.tensor.matmul(ps_s[:], Lt[:, c, 0:1], ksq[:], start=(c == 0), stop=(c == 3))
            nc.tensor.matmul(ps_w[:], Lt[:, c, 1:2], ksq[:], start=(c == 0), stop=(c == 3))
        sq_s = p_sl.tile([1, SL], F32, name="sq_s", tag="sq_s")
        nc.scalar.activation(sq_s[:], ps_s[:], AF.Sqrt, bias=eps_b[0:1, :],
                             scale=1.0 / KV_RANK)
        inv_rms = p_sl.tile([1, SL], F32, name="inv_rms", tag="inv_rms")
        nc.vector.reciprocal(inv_rms[:], sq_s[:])
        tmp_r = p_sl.tile([1, SL], F32, name="tmp_r", tag="tmp_r")
        nc.vector.tensor_copy(tmp_r[:], ps_w[:])
        nc.vector.tensor_mul(tmp_r[:], tmp_r[:], inv_rms[:])
        nc.vector.tensor_mul(tmp_r[:], tmp_r[:], inv_rms[:])
        t_st = p_sl.tile([1, SL], BF16, name="t_st", tag="t_st")
        nc.scalar.activation(t_st[:], tmp_r[:], AF.Sqrt, bias=1.0)

        # broadcast inv_rms via outer product; fused scale -> bf16 stage
        rb = p_pssl.tile([P, SL], F32, name="rb", tag="rb", bufs=1)
        nc.tensor.matmul(rb[:], ones_row[:], inv_rms[:], start=True, stop=True)
        kvn_st = [p_sl.tile([P, SL], BF16, name=f"kvn_st_{c}", tag=f"kvn_st_{c}")
                  for c in range(4)]
        for c in range(4):
            nc.vector.scalar_tensor_tensor(
                kvn_st[c][:], kvsl[c][:], wnormT[:, c:c + 1], rb[:],
                op0=mybir.AluOpType.mult, op1=mybir.AluOpType.mult)

        # rotary on the k_pe slice
        rtl = p_sl.tile([RSP, SL], F32, name="rtl", tag="rtl")
        kpe_st = p_sl.tile([RSP, SL], BF16, name="kpe_st", tag="kpe_st")
        x0 = kpesl[0:32, :]
        x1 = kpesl[32:64, :]
        nc.vector.tensor_mul(rtl[32:64, :], x0, sinsl[0:32, :])
        nc.vector.tensor_mul(rtl[0:32, :], x1, sinsl[32:64, :])
        nc.vector.tensor_mul(x0, x0, cossl[0:32, :])
        nc.vector.tensor_mul(x1, x1, cossl[32:64, :])
        nc.vector.tensor_sub(kpe_st[0:32, :], x0, rtl[0:32, :])
        nc.vector.tensor_add(kpe_st[32:64, :], x1, rtl[32:64, :])

        # ship slice, gather full (single bf16 payload)
        for c in range(4):
            nc.sync.dma_start(out=gin[c * P:(c + 1) * P, :], in_=kvn_st[c][:])
        nc.sync.dma_start(out=gin[KV_RANK:KV_RANK + RSP, :], in_=kpe_st[:])
        nc.sync.dma_start(out=gin[KV_RANK + RSP:, :], in_=t_st[:])
        nc.gpsimd.collective_compute(
            "AllGather", mybir.AluOpType.bypass,
            replica_groups=[list(range(N_CORES))],
            ins=[gin[:]], outs=[gout[:]])
        for c in range(4):
            nc.sync.dma_start(
                out=kv[c][:].rearrange("p (k s) -> p k s", k=N_CORES),
                in_=gout[:, c * P:(c + 1) * P, :].rearrange("k p s -> p k s"))
        nc.sync.dma_start(
            out=kpe[:].rearrange("p (k s) -> p k s", k=N_CORES),
            in_=gout[:, KV_RANK:KV_RANK + RSP, :].rearrange("k p s -> p k s"))
        nc.sync.dma_start(
            out=t_row_bf[:].rearrange("p (k s) -> p k s", k=N_CORES),
            in_=gout[:, KV_RANK + RSP:, :].rearrange("k p s -> p k s"))
        p_pssl.release()
        p_sl.release()

        # --- q projection over the full sequence -----------------------------
        chunks = []
        chunks.append((2 * P, P, _w_qrope))
        for h in range(HPC):
            chunks.append((h * P, P, (lambda h=h: lambda ps, n0, nn:
                nc.scalar.copy(qsA[h][:, n0:n0 + nn], ps[0:P, :nn]))()))

        for n in range(NA):
            xt = p_xs.tile([P, NDC, NCH], BF16, name="xt", tag="xt", bufs=2)
            src = xT_d[:, n * NCH:(n + 1) * NCH].rearrange("(dc p) s -> p dc s", p=P)
            for dc in range(NDC):
                nc.sync.dma_start(out=xt[:, dc, :], in_=src[:, dc, :])
            for (col0, msize, writer) in chunks:
                ps = p_psA.tile([P, NCH], F32, name="psa", tag="psa", bufs=3)
                for dc in range(NDC):
                    nc.tensor.matmul(ps[:msize, :], wQ[dc][:, col0:col0 + msize],
                                     xt[:, dc, :],
                                     start=(dc == 0), stop=(dc == NDC - 1))
                writer(ps, n * NCH, NCH)
        p_psA.release()
        p_xs.release()
        p_wA.release()

        # ---------------- Phase A2: rotary + RMS norm ------------------------
        p_a2 = tc.alloc_tile_pool(name="p_a2", bufs=1)
        p_ps2 = tc.alloc_tile_pool(name="p_ps2", bufs=2, space="PSUM")

        # tables are host-duplicated to [64, S] (rows 32..63 repeat 0..31) so
        # every TensorTensor has equal input base partitions
        cosT = p_a2.tile([RSP, S], F32, name="cosT", tag="cosT")
        sinT = p_a2.tile([RSP, S], F32, name="sinT", tag="sinT")
        nc.sync.dma_start(out=cosT[:], in_=cosT_d[:])
        nc.sync.dma_start(out=sinT[:], in_=sinT_d[:])

        # rotary (pairs split even/odd by host permutation): x0 rows 0..31,
        # x1 rows 32..63 of each 64-row rope group
        rt = p_a2.tile([RSP, S], F32, name="rt", tag="rt")
        for grp in [qsB[0], qsB[1]]:
            x0 = grp[0:32, :]
            x1 = grp[32:64, :]
            nc.vector.tensor_mul(rt[32:64, :], x0, sinT[0:32, :])   # x0*s -> hi
            nc.vector.tensor_mul(rt[0:32, :], x1, sinT[32:64, :])   # x1*s -> lo
            nc.vector.tensor_mul(x0, x0, cosT[0:32, :])             # x0*c
            nc.vector.tensor_mul(x1, x1, cosT[32:64, :])            # x1*c
            nc.vector.tensor_sub(x0, x0, rt[0:32, :])               # y0
            nc.vector.tensor_add(x1, x1, rt[32:64, :])              # y1
        p_ps2.release()
        p_a2.release()

        # ---------------- Phase A3: kv_b projection --------------------------
        big2 = tc.alloc_tile_pool(name="big2", bufs=1)
        p_wB = tc.alloc_tile_pool(name="p_wB", bufs=1)
        p_psB = tc.alloc_tile_pool(name="p_psB", bufs=4, space="PSUM")
        wb_k = []
        for k in range(4):
            w = p_wB.tile([P, WB_COLS], BF16, name=f"wbk_{k}", tag=f"wbk_{k}")
            nc.sync.dma_start(out=w[:], in_=wkvb_d[k * P:(k + 1) * P, :])
            wb_k.append(w)
        wb_t = p_wB.tile([1, WB_COLS], BF16, name="wb_t", tag="wb_t")
        nc.sync.dma_start(out=wb_t[:], in_=wkvb_d[KV_RANK:KV_RANK + 1, :])

        ksA = [big2.tile([P, S], F32R, name=f"ksA_{h}", tag=f"ksA_{h}") for h in range(HPC)]
        bigsqs = [big2.tile([P, S], F32, name=f"bigsq_{h}", tag=f"bigsq_{h}")
                  for h in range(HPC)]
        vts = [big2.tile([VSP, S], F32, name=f"vts_{h}", tag=f"vts_{h}") for h in range(HPC)]

        bchunks = []
        for h in range(HPC):
            bchunks.append((h * (NOPE + VSP), P,
                            (lambda h=h: lambda n0, nn: ksA[h][:, n0:n0 + nn])(),
                            (lambda h=h: lambda ps, n0, nn:
                             nc.scalar.square(bigsqs[h][:, n0:n0 + nn], ps[0:P, :nn]))()))
            bchunks.append((h * (NOPE + VSP) + NOPE, VSP,
                            (lambda h=h: lambda n0, nn: vts[h][:, n0:n0 + nn])(), None))

        for n in range(4):
            n0 = n * 512
            for (col0, msize, dst, extra) in bchunks:
                ps = p_psB.tile([P, 512], F32, name="psb", tag="psb", bufs=4)
                for k in range(4):
                    nc.tensor.matmul(ps[:msize, :], wb_k[k][:, col0:col0 + msize],
                                     kv[k][:, n0:n0 + 512], start=(k == 0), stop=False)
                nc.tensor.matmul(ps[:msize, :], wb_t[:, col0:col0 + msize],
                                 t_row_bf[:, n0:n0 + 512], start=False, stop=True)
                nc.scalar.copy(dst(n0, 512), ps[:msize, :])
                if extra is not None:
                    extra(ps, n0, 512)

        p_psB.release()
        p_wB.release()

        # ---------------- Phase B: time rows + V' assembly -------------------
        p_sqB = tc.alloc_tile_pool(name="p_sqB", bufs=1)
        p_ps1 = tc.alloc_tile_pool(name="p_ps1", bufs=1, space="PSUM")
        p_psT = tc.alloc_tile_pool(name="p_psT", bufs=2, space="PSUM")

        # V' transposes first: they only need vts, so they overlap the squares
        for h in range(HPC):
            for j in range(NQT):
                tp = p_psT.tile([P, P], F32, name="tp", tag="tp", bufs=2)
                nc.tensor.transpose(tp[:VSP + 1, :VSP], vts[h][:, j * P:(j + 1) * P],
                                    identity[0:VSP, 0:VSP])
                nc.vector.tensor_copy(Vp[h][:, j, 0:VSP], tp[:, 0:VSP])
            # batched time column for all 16 V' tiles
            vsq = p_sqB.tile([P, NQT, VSP], F32, name="vsq", tag="vsq", bufs=1)
            nc.scalar.square(vsq[:], Vp[h][:, :, 0:VSP])
            vsum = p_sqB.tile([P, NQT, 1], F32, name="vsum", tag="vsum", bufs=1)
            nc.vector.reduce_sum(vsum[:], vsq[:], axis=AX.X)
            nc.scalar.activation(Vp[h][:, :, VSP:VSP + 1], vsum[:], AF.Sqrt, bias=1.0)

        kpesq = p_sqB.tile([RSP, S], F32, name="kpesq", tag="kpesq")
        nc.scalar.square(kpesq[:], kpe[:])
        for h in range(HPC):
            # k_pe rows are shared by both heads
            nc.gpsimd.tensor_copy(ksB[h][0:RSP, :], kpe[:])

            bigsq = bigsqs[h]
            smsq = p_sqB.tile([RSP, S], F32, name="smsq", tag="smsq", bufs=1)
            nc.scalar.square(smsq[:], qsB[h][0:RSP, :])
            qbigsq = p_sqB.tile([P, S], F32, name="qbigsq", tag="qbigsq", bufs=2)
            nc.scalar.square(qbigsq[:], qsA[h][:])
            qtmp = p_sqB.tile([1, S], F32, name="qtmp", tag="qtmp")
            for n in range(4):
                n0 = n * 512
                pk = p_ps1.tile([1, 512], F32, name="pk", tag="pk", bufs=2)
                nc.tensor.matmul(pk[:], ones_col[:], bigsq[:, n0:n0 + 512],
                                 start=True, stop=False)
                nc.tensor.matmul(pk[:], ones_col[0:RSP, :], kpesq[:, n0:n0 + 512],
                                 start=False, stop=True)
                nc.scalar.activation(ksB[h][RSP:RSP + 1, n0:n0 + 512], pk[:],
                                     AF.Sqrt, bias=1.0)
                pq = p_ps1.tile([1, 512], F32, name="pq", tag="pq", bufs=2)
                nc.tensor.matmul(pq[:], ones_col[:], qbigsq[:, n0:n0 + 512],
                                 start=True, stop=False)
                nc.tensor.matmul(pq[:], ones_col[0:RSP, :], smsq[:, n0:n0 + 512],
                                 start=False, stop=True)
                nc.scalar.activation(qtmp[:, n0:n0 + 512], pq[:], AF.Sqrt, bias=1.0)
            nc.vector.tensor_scalar_mul(qsB[h][RSP:RSP + 1, :], qtmp[:], -1.0)

        p_psT.release()
        p_ps1.release()
        p_sqB.release()

        # ---------------- Phase C: attention ---------------------------------
        # scoresT layout [k, q]: lhsT and rhs are both [d, s] slices; exp(PSUM)
        # lands in SBUF already transposed for the AV matmul. The centroid
        # epilogue normalizes the raw exp-weighted sum (softmax denominator
        # cancels), then PE-transposes cen for the wo matmul.
        cenT = [[big.tile([P, P], BF16, name=f"cenT_{h}_{m}", tag=f"cenT_{h}_{m}")
                 for m in range(NQT)] for h in range(HPC)]
        GQ = 512 // P
        NG = S // 512
        p_ex = tc.alloc_tile_pool(name="p_ex", bufs=4)
        p_cw = tc.alloc_tile_pool(name="p_cw", bufs=2)
        p_wO = tc.alloc_tile_pool(name="p_wO", bufs=1)
        p_osb = tc.alloc_tile_pool(name="p_osb", bufs=4)
        p_ave = tc.alloc_tile_pool(name="p_ave", bufs=1, space="PSUM")
        p_scp = tc.alloc_tile_pool(name="p_scp", bufs=3, space="PSUM")
        p_ptc = tc.alloc_tile_pool(name="p_ptc", bufs=1, space="PSUM")
        p_psD = tc.alloc_tile_pool(name="p_psD", bufs=3, space="PSUM")

        wo_sb = []
        for h in range(HPC):
            w = p_wO.tile([P, OUT_COLS], BF16, name=f"wo_{h}", tag=f"wo_{h}")
            nc.sync.dma_start(out=w[:], in_=wo_d[h * P:(h + 1) * P, :])
            wo_sb.append(w)

        for g in range(NG):
            for h in range(HPC):
                # all four AV accumulators packed into one PSUM bank
                ave = p_ave.tile([P, GQ, P], F32, name="ave", tag="ave", bufs=2)
                jmax = (g * GQ + GQ) if causal else NQT
                for j in range(jmax):
                    lo = max(0, j - g * GQ) if causal else 0
                    ncols = (GQ - lo) * P
                    c0 = g * 512 + lo * P
                    sc = p_scp.tile([P, 512], F32, name="sc", tag="sc", bufs=3)
                    nc.tensor.matmul(sc[:, :ncols], ksA[h][:, j * P:(j + 1) * P],
                                     qsA[h][:, c0:c0 + ncols], start=True, stop=False)
                    nc.tensor.matmul(sc[:, :ncols], ksB[h][:, j * P:(j + 1) * P],
                                     qsB[h][:, c0:c0 + ncols], start=False, stop=True)
                    ex = p_ex.tile([P, 512], F32R, name="ex", tag="ex", bufs=6)
                    nc.scalar.activation(ex[:, :ncols], sc[:, :ncols], AF.Exp,
                                         scale=exp_scale)
                    if causal and j >= g * GQ:
                        nc.vector.tensor_mul(ex[:, 0:P], ex[:, 0:P], diagmask[:])
                    for t in range(lo, GQ):
                        qt_idx = g * GQ + t
                        # start=True clears the WHOLE bank, so only the first
                        # matmul of the packed accumulator issues it; the other
                        # lanes overwrite-on-cleared-has_written at j == 0.
                        nc.tensor.matmul(ave[:, t, :], ex[:, (t - lo) * P:(t - lo + 1) * P],
                                         Vp[h][:, j, :], start=(j == 0 and t == 0),
                                         stop=(j == (qt_idx if causal else NQT - 1)),
                                         skip_group_check=True)
                for t in range(GQ):
                    qt_idx = g * GQ + t
                    sqt = p_cw.tile([P, P], F32, name="sqt", tag="sqt", bufs=2)
                    nc.scalar.square(sqt[:], ave[:, t, :])
                    rsum = p_cw.tile([P, 1], F32, name="rsum", tag="rsum", bufs=2)
                    nc.vector.reduce_sum(rsum[:], sqt[:], axis=AX.X)
                    tsq0 = p_cw.tile([P, 1], F32, name="tsq0", tag="tsq0", bufs=2)
                    nc.vector.tensor_scalar_mul(tsq0[:], sqt[:, VSP:VSP + 1], 2.0)
                    inner = p_cw.tile([P, 1], F32, name="inner", tag="inner", bufs=2)
                    nc.vector.tensor_sub(inner[:], rsum[:], tsq0[:])
                    nc.scalar.activation(inner[:], inner[:], AF.Abs)
                    nc.scalar.activation(inner[:], inner[:], AF.Sqrt)
                    rs = p_cw.tile([P, 1], F32, name="rs", tag="rs", bufs=2)
                    nc.vector.reciprocal(rs[:], inner[:])
                    cen = p_cw.tile([P, P], F32, name="cen", tag="cen", bufs=2)
                    nc.vector.tensor_scalar_mul(cen[:], ave[:, t, :], rs[:])
                    tpc = p_ptc.tile([P, P], F32, name="tpc", tag="tpc", bufs=1)
                    nc.tensor.transpose(tpc[:], cen[:], identity[:])
                    nc.any.tensor_copy(cenT[h][qt_idx][:], tpc[:])
            # wo projection for this group's q-tiles (both heads done)
            for t in range(GQ):
                m = g * GQ + t
                for n in range(4):
                    n0 = n * 512
                    nn = min(512, OUT_COLS - n0)
                    ps = p_psD.tile([P, 512], F32, name="psd", tag="psd", bufs=2)
                    nc.tensor.matmul(ps[:, :nn], cenT[0][m][:],
                                     wo_sb[0][:, n0:n0 + nn], start=True, stop=False)
                    nc.tensor.matmul(ps[:, :nn], cenT[1][m][:],
                                     wo_sb[1][:, n0:n0 + nn], start=False, stop=True)
                    ot = p_osb.tile([P, 512], F32, name="ot", tag="ot", bufs=6)
                    nc.any.tensor_copy(ot[:, :nn], ps[:, :nn])
                    nc.sync.dma_start(out=out_d[m * P:(m + 1) * P, n0:n0 + nn],
                                      in_=ot[:, :nn])

        p_psD.release()
        p_ptc.release()
        p_scp.release()
        p_ave.release()
        p_osb.release()
        p_wO.release()
        p_cw.release()
        p_ex.release()

        big2.release()
        big.release()
        const.release()

    nc.compile()
    return nc


_CACHE = {}


def _get_program(exp_scale: float, causal: bool):
    key = (round(float(exp_scale), 12), causal)
    if key not in _CACHE:
        _CACHE[key] = _build_program(float(exp_scale), causal)
    return _CACHE[key]


def _rope_perm():
    """Even rope dims first, then odd (host-side column permutation)."""
    return np.concatenate([np.arange(0, RSP, 2), np.arange(1, RSP, 2)])


def kernel(x, start_pos, freqs_cos, freqs_sin, mask, wq_w, wkv_a_w, kv_norm_w,
           wkv_b_w, wo_w, softmax_scale, bias_p, _want_trace=False):
    x2 = np.ascontiguousarray(np.asarray(x, np.float32).reshape(S, DIM))
    xT = np.ascontiguousarray(x2.T)
    wq_w = np.asarray(wq_w, np.float32)
    wkv_a_w = np.asarray(wkv_a_w, np.float32)
    kv_norm_w = np.asarray(kv_norm_w, np.float32)
    wkv_b_w = np.asarray(wkv_b_w, np.float32)
    wo_w = np.asarray(wo_w, np.float32)
    cosT = np.asarray(freqs_cos, np.float32).T
    sinT = np.asarray(freqs_sin, np.float32).T
    cosT = np.ascontiguousarray(np.concatenate([cosT, cosT], axis=0))
    sinT = np.ascontiguousarray(np.concatenate([sinT, sinT], axis=0))

    mask = np.asarray(mask)
    causal = bool(np.array_equal(mask, np.triu(np.ones((S, S), bool), k=1)))
    if not causal:
        assert not mask.any(), "only causal or empty masks are supported"

    smax = float(np.asarray(softmax_scale).reshape(-1)[0])
    exp_scale = 2.0 / smax

    rp = _rope_perm()
    # wq per core-pair layout: [nope_h0 | nope_h1 | rope_h0(ev,od) | rope_h1(ev,od)]
    wq_r = wq_w.reshape(DIM, 16, QH)
    wq_nope = wq_r[:, :, :NOPE]                       # (DIM, 16, 128)
    wq_rope = wq_r[:, :, NOPE:][:, :, rp]             # (DIM, 16, 64) permuted
    wq_cores = []
    for c in range(N_CORES):
        h0, h1 = 2 * c, 2 * c + 1
        wq_cores.append(np.concatenate(
            [wq_nope[:, h0], wq_nope[:, h1], wq_rope[:, h0], wq_rope[:, h1]],
            axis=1))
    # wkva: [kv | rope-even | rope-odd]
    wkva_p = wkv_a_w.copy()
    wkva_p[:, KV_RANK:] = wkva_p[:, KV_RANK:][:, rp]
    # wkvb: kvn rows first, time row last
    wkvb_p = np.ascontiguousarray(np.concatenate([wkv_b_w[1:], wkv_b_w[:1]], axis=0))
    wnormT = np.ascontiguousarray(kv_norm_w.reshape(4, P).T)
    # wo rows per head: [v space (1..127), time (0)]
    wo_p = wo_w.reshape(16, P, OUT_COLS)
    wo_p = np.concatenate([wo_p[:, 1:, :], wo_p[:, 0:1, :]], axis=1)
    wo_p = wo_p.reshape(16 * P, OUT_COLS)

    nc = _get_program(exp_scale, causal)

    xT_bf = np.ascontiguousarray(xT.astype(ml_dtypes.bfloat16))
    wkva_bf = np.ascontiguousarray(wkva_p.astype(ml_dtypes.bfloat16))
    SL = S // N_CORES

    in_maps = []
    for c in range(N_CORES):
        in_maps.append({
            "xT": xT_bf,
            "wq": np.ascontiguousarray(wq_cores[c].astype(ml_dtypes.bfloat16)),
            "wkva": wkva_bf,
            "wnormT": wnormT,
            "wkvb": np.ascontiguousarray(
                wkvb_p[:, c * WB_COLS:(c + 1) * WB_COLS].astype(ml_dtypes.bfloat16)),
            "wo": np.ascontiguousarray(
                wo_p[c * WO_ROWS:(c + 1) * WO_ROWS, :].astype(ml_dtypes.bfloat16)),
            "cosT": cosT,
            "sinT": sinT,
            "xsl": np.ascontiguousarray(xT_bf[:, c * SL:(c + 1) * SL]),
            "cossl": np.ascontiguousarray(cosT[:, c * SL:(c + 1) * SL]),
            "sinsl": np.ascontiguousarray(sinT[:, c * SL:(c + 1) * SL]),
        })

    res = run_bass_kernel_spmd(nc, in_maps, core_ids=list(range(N_CORES)),
                               trace=_want_trace)
    kernel.last_result = res

    total = res.results[0]["out"].astype(np.float32)
    for c in range(1, N_CORES):
        total = total + res.results[c]["out"]
    t = np.sqrt(np.sum(total * total, axis=-1, keepdims=True) + 1.0)
    out = np.concatenate([t, total], axis=-1)
    return out.reshape(1, S, DIM).astype(np.float32)



# revision 15
# speedup vs baseline: 1.1410x; 1.1410x over previous
"""Lorentz MLA attention kernel for Trainium2, sharded over 8 NeuronCores.

Sharding: tensor-parallel over the 16 attention heads (2 heads per core);
the kv_lora latent projection (wkv_a + RMS norm) is sequence-sharded and
AllGathered. The output projection wo is row-parallel: each core produces a
partial (2048, 2047) output in bf16; the host sums the 8 partials in f32 and
applies the final Lorentz lift.

Device-side layout notes:
- Everything flows transposed ([feature, s]) so all matmuls contract on the
  partition axis without transposing x on device (host passes x^T).
- The 193-dim Lorentz q/k contraction is chunked [128 nope] + [64 rope + 1
  time]; the time rows sit at partition 64 of the 65-row "B" tiles.
- Rope dims are permuted even-pairs-first on the host so rotary is aligned
  [32, n] ops; weight columns are permuted to match.
- All matmul operands are bf16 (f32r at <256 moving columns runs at 1/4 PE
  rate); PSUM accumulation stays f32.
- Scalar engine uses ONLY the natural_log_exp activation table: every sqrt
  is computed as exp(0.5*ln(1+x)) so no ACT table reloads ever happen.
- Softmax max-pass skipped (scores <= 0 on the hyperboloid) and the softmax
  denominator cancels inside the Lorentz centroid normalization. V' carries
  its time coordinate in column 127 (wo rows are permuted on host to match).
- The attention j-loop is software-pipelined (scores j+2 issued before the
  exp-gated AV matmul j) so the PE never idles waiting on the scalar engine.
- V' tiles are produced with DMA xbar transposes, off the PE.
"""

import os
import sys
import types

import numpy as np
import ml_dtypes


def _ensure_axon_hooks():
    """Recreate the missing antenv.axon_hooks module so NTFF tracing works."""
    if "antenv.axon_hooks" in sys.modules:
        return
    try:
        import antenv
        from trn_agent_boot.trn_boot import _ntff_profile_via_ctypes

        hook = _ntff_profile_via_ctypes("/opt/axon/libaxon_pjrt.so")
        mod = types.ModuleType("antenv.axon_hooks")
        mod.get_axon_ntff_profile_hook = lambda: hook
        mod.set_axon_ntff_profile_hook = lambda h: None
        sys.modules["antenv.axon_hooks"] = mod
        antenv.axon_hooks = mod
    except Exception:
        pass


_ensure_axon_hooks()

import concourse.bacc as bacc
import concourse.bass as bass
import concourse.tile as tile
from concourse import mybir
import concourse.bass_utils as bass_utils
from concourse.bass_utils import run_bass_kernel_spmd
from concourse.masks import make_identity, make_upper_triangular

# zero-egress container: make the S3 artifact upload in the profile path a no-op
bass_utils.upload_artifacts = lambda tmpdir: tmpdir

F32 = mybir.dt.float32
BF16 = mybir.dt.bfloat16
AF = mybir.ActivationFunctionType
AX = mybir.AxisListType
ALU = mybir.AluOpType

N_CORES = 8
P = 128
S = 2048          # sequence length
DIM = 2048        # model dim
NDC = DIM // P    # 16 contraction chunks over DIM
NQT = S // P      # 16 q/k tiles of 128
HPC = 2           # heads per core
NOPE = 128
RSP = 64          # rotary space dim
VSP = 127         # v space dim
KV_RANK = 512
EPS_RMS = 1e-6
QH = NOPE + RSP               # 192 q space rows per head
WQ_COLS = HPC * QH            # 384
WB_COLS = HPC * (NOPE + VSP)  # 510
WO_ROWS = HPC * P             # 256
OUT_COLS = DIM - 1            # 2047
NCH = 512                     # column chunk
NA = S // NCH                 # 4 chunks
SL = S // N_CORES             # 256
GR = KV_RANK + RSP + 1        # gathered rows: kvn + kpe + t_row


def _build_program(exp_scale: float, causal: bool):
    nc = bacc.Bacc("TRN2", target_bir_lowering=False, debug=False,
                   num_devices=N_CORES)

    xT_d = nc.dram_tensor("xT", [DIM, S], BF16, kind="ExternalInput")
    wq_d = nc.dram_tensor("wq", [DIM, WQ_COLS], BF16, kind="ExternalInput")
    wkva_d = nc.dram_tensor("wkva", [DIM, KV_RANK + RSP], BF16, kind="ExternalInput")
    wnormT_d = nc.dram_tensor("wnormT", [P, 4], F32, kind="ExternalInput")
    wkvb_d = nc.dram_tensor("wkvb", [KV_RANK + 1, WB_COLS], BF16, kind="ExternalInput")
    wo_d = nc.dram_tensor("wo", [WO_ROWS, OUT_COLS], BF16, kind="ExternalInput")
    cosT_d = nc.dram_tensor("cosT", [RSP, S], BF16, kind="ExternalInput")
    sinT_d = nc.dram_tensor("sinT", [RSP, S], BF16, kind="ExternalInput")
    out_d = nc.dram_tensor("out", [S, OUT_COLS], BF16, kind="ExternalOutput")
    xsl_d = nc.dram_tensor("xsl", [DIM, SL], BF16, kind="ExternalInput")
    cossl_d = nc.dram_tensor("cossl", [RSP, SL], BF16, kind="ExternalInput")
    sinsl_d = nc.dram_tensor("sinsl", [RSP, SL], BF16, kind="ExternalInput")
    gin = nc.dram_tensor("gin", [GR, SL], BF16)
    gout = nc.dram_tensor("gout", [N_CORES, GR, SL], BF16, addr_space="Shared")

    with tile.TileContext(nc) as tc:
        const = tc.alloc_tile_pool(name="const", bufs=1)
        identity = const.tile([P, P], BF16)
        make_identity(nc, identity)
        diagmask = const.tile([P, P], BF16)
        make_upper_triangular(nc, diagmask, val=1.0, diag=True)
        wnormT = const.tile([P, 4], F32)
        nc.sync.dma_start(out=wnormT[:], in_=wnormT_d[:])
        Lt = const.tile([P, 4, 2], BF16)  # [ones | wnorm^2] per latent chunk
        for c in range(4):
            nc.vector.memset(Lt[:, c, 0:1], 1.0)
            nc.vector.tensor_mul(Lt[:, c, 1:2], wnormT[:, c:c + 1], wnormT[:, c:c + 1])
        ones_col = const.tile([P, 1], BF16)
        nc.vector.memset(ones_col[:], 1.0)
        ones_row = const.tile([1, P], F32)
        nc.vector.memset(ones_row[:], 1.0)
        eps_b = const.tile([P, 1], F32)
        nc.vector.memset(eps_b[:], EPS_RMS)

        # Long-lived tiles.
        big = tc.alloc_tile_pool(name="big", bufs=1)
        qsA = [big.tile([P, S], BF16, name=f"qsA_{h}", tag=f"qsA_{h}") for h in range(HPC)]
        qsB = [big.tile([RSP + 1, S], BF16, name=f"qsB_{h}", tag=f"qsB_{h}") for h in range(HPC)]
        kv = [big.tile([P, S], BF16, name=f"kv_{c}", tag=f"kv_{c}") for c in range(4)]
        kpe = big.tile([RSP, S], BF16, name="kpe", tag="kpe")
        ksB = [big.tile([RSP + 1, S], BF16, name=f"ksB_{h}", tag=f"ksB_{h}") for h in range(HPC)]
        Vp = [big.tile([P, NQT, P], BF16, name=f"Vp_{h}", tag=f"Vp_{h}") for h in range(HPC)]
        t_row_bf = big.tile([1, S], BF16, name="t_row_bf", tag="t_row_bf")

        # ------------- Slice phase: kv latent on this core's s-slice ---------
        p_wKV = tc.alloc_tile_pool(name="p_wKV", bufs=1)
        p_sl = tc.alloc_tile_pool(name="p_sl", bufs=1)
        p_pssl = tc.alloc_tile_pool(name="p_pssl", bufs=2, space="PSUM")
        wKV = []
        for dc in range(NDC):
            w = p_wKV.tile([P, KV_RANK + RSP], BF16, name=f"wKV_{dc}", tag=f"wKV_{dc}")
            nc.sync.dma_start(out=w[:], in_=wkva_d[dc * P:(dc + 1) * P, :])
            wKV.append(w)
        xsl_t = p_sl.tile([P, NDC, SL], BF16, name="xsl_t", tag="xsl_t")
        for dc in range(NDC):
            nc.sync.dma_start(out=xsl_t[:, dc, :],
                              in_=xsl_d[dc * P:(dc + 1) * P, :])
        cossl = p_sl.tile([RSP, SL], BF16, name="cossl", tag="cossl")
        sinsl = p_sl.tile([RSP, SL], BF16, name="sinsl", tag="sinsl")
        nc.sync.dma_start(out=cossl[:], in_=cossl_d[:])
        nc.sync.dma_start(out=sinsl[:], in_=sinsl_d[:])

        kvsl = [p_sl.tile([P, SL], F32, name=f"kvsl_{c}", tag=f"kvsl_{c}")
                for c in range(4)]
        kpesl = p_sl.tile([RSP, SL], F32, name="kpesl", tag="kpesl")
        for c in range(4):
            ps = p_pssl.tile([P, SL], F32, name="psl", tag="psl", bufs=2)
            for dc in range(NDC):
                nc.tensor.matmul(ps[:], wKV[dc][:, c * P:(c + 1) * P],
                                 xsl_t[:, dc, :], start=(dc == 0), stop=(dc == NDC - 1))
            nc.vector.tensor_copy(kvsl[c][:], ps[:])
        ps = p_pssl.tile([P, SL], F32, name="psl", tag="psl", bufs=2)
        for dc in range(NDC):
            nc.tensor.matmul(ps[:RSP, :], wKV[dc][:, KV_RANK:],
                             xsl_t[:, dc, :], start=(dc == 0), stop=(dc == NDC - 1))
        nc.vector.tensor_copy(kpesl[:], ps[:RSP, :])

        # RMS stats on the slice
        ps_s = p_pssl.tile([1, SL], F32, name="ps_s", tag="ps_s", bufs=1)
        ps_w = p_pssl.tile([1, SL], F32, name="ps_w", tag="ps_w", bufs=1)
        for c in range(4):
            ksq = p_sl.tile([P, SL], BF16, name="ksq", tag="ksq", bufs=2)
            nc.scalar.square(ksq[:], kvsl[c][:])
            nc.tensor.matmul(ps_s[:], Lt[:, c, 0:1], ksq[:], start=(c == 0), stop=(c == 3))
            nc.tensor.matmul(ps_w[:], Lt[:, c, 1:2], ksq[:], start=(c == 0), stop=(c == 3))
        # inv_rms = exp(-0.5 * ln(mean_sq + eps)) ; single ACT table (ln/exp)
        ln_s = p_sl.tile([1, SL], F32, name="ln_s", tag="ln_s")
        nc.scalar.activation(ln_s[:], ps_s[:], AF.Ln, bias=eps_b[0:1, :],
                             scale=1.0 / KV_RANK)
        inv_rms = p_sl.tile([1, SL], F32, name="inv_rms", tag="inv_rms")
        nc.scalar.activation(inv_rms[:], ln_s[:], AF.Exp, scale=-0.5)
        tmp_r = p_sl.tile([1, SL], F32, name="tmp_r", tag="tmp_r")
        nc.vector.tensor_copy(tmp_r[:], ps_w[:])
        nc.vector.tensor_mul(tmp_r[:], tmp_r[:], inv_rms[:])
        nc.vector.tensor_mul(tmp_r[:], tmp_r[:], inv_rms[:])
        t_ln = p_sl.tile([1, SL], F32, name="t_ln", tag="t_ln")
        nc.scalar.activation(t_ln[:], tmp_r[:], AF.Ln, bias=1.0)
        t_st = p_sl.tile([1, SL], BF16, name="t_st", tag="t_st")
        nc.scalar.activation(t_st[:], t_ln[:], AF.Exp, scale=0.5)

        # broadcast inv_rms via outer product; fused scale -> bf16 stage
        rb = p_pssl.tile([P, SL], F32, name="rb", tag="rb", bufs=1)
        nc.tensor.matmul(rb[:], ones_row[:], inv_rms[:], start=True, stop=True)
        kvn_st = [p_sl.tile([P, SL], BF16, name=f"kvn_st_{c}", tag=f"kvn_st_{c}")
                  for c in range(4)]
        for c in range(4):
            nc.vector.scalar_tensor_tensor(
                kvn_st[c][:], kvsl[c][:], wnormT[:, c:c + 1], rb[:],
                op0=ALU.mult, op1=ALU.mult)

        # rotary on the k_pe slice
        rtl = p_sl.tile([RSP, SL], F32, name="rtl", tag="rtl")
        kpe_st = p_sl.tile([RSP, SL], BF16, name="kpe_st", tag="kpe_st")
        x0 = kpesl[0:32, :]
        x1 = kpesl[32:64, :]
        nc.vector.tensor_mul(rtl[32:64, :], x0, sinsl[0:32, :])
        nc.vector.tensor_mul(rtl[0:32, :], x1, sinsl[32:64, :])
        nc.vector.tensor_mul(x0, x0, cossl[0:32, :])
        nc.vector.tensor_mul(x1, x1, cossl[32:64, :])
        nc.vector.tensor_sub(kpe_st[0:32, :], x0, rtl[0:32, :])
        nc.vector.tensor_add(kpe_st[32:64, :], x1, rtl[32:64, :])

        # ship slice, gather full (single bf16 payload)
        for c in range(4):
            nc.sync.dma_start(out=gin[c * P:(c + 1) * P, :], in_=kvn_st[c][:])
        nc.sync.dma_start(out=gin[KV_RANK:KV_RANK + RSP, :], in_=kpe_st[:])
        nc.sync.dma_start(out=gin[KV_RANK + RSP:, :], in_=t_st[:])
        nc.gpsimd.collective_compute(
            "AllGather", ALU.bypass,
            replica_groups=[list(range(N_CORES))],
            ins=[gin[:]], outs=[gout[:]])
        for c in range(4):
            nc.sync.dma_start(
                out=kv[c][:].rearrange("p (k s) -> p k s", k=N_CORES),
                in_=gout[:, c * P:(c + 1) * P, :].rearrange("k p s -> p k s"))
        nc.sync.dma_start(
            out=kpe[:].rearrange("p (k s) -> p k s", k=N_CORES),
            in_=gout[:, KV_RANK:KV_RANK + RSP, :].rearrange("k p s -> p k s"))
        nc.sync.dma_start(
            out=t_row_bf[:].rearrange("p (k s) -> p k s", k=N_CORES),
            in_=gout[:, KV_RANK + RSP:, :].rearrange("k p s -> p k s"))
        p_pssl.release()
        p_sl.release()

        # k_pe rows are shared by both heads; copy on the (idle) gpsimd
        for h in range(HPC):
            nc.gpsimd.tensor_copy(ksB[h][0:RSP, :], kpe[:])

        # --- Phase A: q projection over the full sequence --------------------
        # weight col layout (host): [qnope0 | qnope1 | qrope0(ev,od) | qrope1]
        # Per n-chunk: matmuls, drains to bf16, rotary (vector), q-time rows.
        p_wA = tc.alloc_tile_pool(name="p_wA", bufs=1)
        p_xs = tc.alloc_tile_pool(name="p_xs", bufs=1)
        p_psA = tc.alloc_tile_pool(name="p_psA", bufs=3, space="PSUM")
        p_pq = tc.alloc_tile_pool(name="p_pq", bufs=2, space="PSUM")
        p_qsc = tc.alloc_tile_pool(name="p_qsc", bufs=1)
        cosT = p_qsc.tile([RSP, S], BF16, name="cosT", tag="cosT")
        sinT = p_qsc.tile([RSP, S], BF16, name="sinT", tag="sinT")
        nc.sync.dma_start(out=cosT[:], in_=cosT_d[:])
        nc.sync.dma_start(out=sinT[:], in_=sinT_d[:])
        wQ = []
        for dc in range(NDC):
            w = p_wA.tile([P, WQ_COLS], BF16, name=f"wQ_{dc}", tag=f"wQ_{dc}")
            nc.sync.dma_start(out=w[:], in_=wq_d[dc * P:(dc + 1) * P, :])
            wQ.append(w)

        for n in range(NA):
            n0 = n * NCH
            xt = p_xs.tile([P, NDC, NCH], BF16, name="xt", tag="xt", bufs=2)
            src = xT_d[:, n0:n0 + NCH].rearrange("(dc p) s -> p dc s", p=P)
            for dc in range(NDC):
                nc.sync.dma_start(out=xt[:, dc, :], in_=src[:, dc, :])

            # rope chunk for both heads: rows [h0ev|h0od|h1ev|h1od]
            ps = p_psA.tile([P, NCH], F32, name="psa", tag="psa", bufs=3)
            for dc in range(NDC):
                nc.tensor.matmul(ps[:], wQ[dc][:, 2 * P:3 * P], xt[:, dc, :],
                                 start=(dc == 0), stop=(dc == NDC - 1))
            qsq = p_qsc.tile([P, NCH], BF16, name="qsq", tag="qsq", bufs=2)
            nc.scalar.square(qsq[:], ps[:])   # rotation preserves rope norms
            for h in range(HPC):
                nc.scalar.copy(qsB[h][0:RSP, n0:n0 + NCH], ps[h * RSP:(h + 1) * RSP, :])
            # rotary, in place on bf16 (2x DVE mode)
            rt = p_qsc.tile([RSP, NCH], BF16, name="rt", tag="rt", bufs=2)
            for h in range(HPC):
                gx0 = qsB[h][0:32, n0:n0 + NCH]
                gx1 = qsB[h][32:64, n0:n0 + NCH]
                nc.vector.tensor_mul(rt[32:64, :], gx0, sinT[0:32, n0:n0 + NCH])
                nc.vector.tensor_mul(rt[0:32, :], gx1, sinT[32:64, n0:n0 + NCH])
                nc.vector.tensor_mul(gx0, gx0, cosT[0:32, n0:n0 + NCH])
                nc.vector.tensor_mul(gx1, gx1, cosT[32:64, n0:n0 + NCH])
                nc.vector.tensor_sub(gx0, gx0, rt[0:32, :])
                nc.vector.tensor_add(gx1, gx1, rt[32:64, :])

            qbsqs = []
            for h in range(HPC):
                ps = p_psA.tile([P, NCH], F32, name="psa", tag="psa", bufs=3)
                for dc in range(NDC):
                    nc.tensor.matmul(ps[:], wQ[dc][:, h * P:(h + 1) * P],
                                     xt[:, dc, :], start=(dc == 0), stop=(dc == NDC - 1))
                nc.vector.tensor_copy(qsA[h][:, n0:n0 + NCH], ps[:])
                qbsq = p_qsc.tile([P, NCH], BF16, name="qbsq", tag="qbsq", bufs=2)
                nc.scalar.square(qbsq[:], ps[:])
                qbsqs.append(qbsq)
            # q time rows: -sqrt(1 + |qnope|^2 + |qrope|^2); emitted after
            # both heads' nope matmuls so the squares are ready (no PE stall)
            for h in range(HPC):
                pq = p_pq.tile([1, NCH], F32, name="pq", tag="pq", bufs=2)
                nc.tensor.matmul(pq[:], ones_col[:], qbsqs[h][:],
                                 start=True, stop=False)
                nc.tensor.matmul(pq[:], ones_col[h * RSP:(h + 1) * RSP, :],
                                 qsq[h * RSP:(h + 1) * RSP, :],
                                 start=False, stop=True)
                qln = p_qsc.tile([1, NCH], F32, name="qln", tag="qln", bufs=2)
                nc.scalar.activation(qln[:], pq[:], AF.Ln, bias=1.0)
                qex = p_qsc.tile([1, NCH], F32, name="qex", tag="qex", bufs=2)
                nc.scalar.activation(qex[:], qln[:], AF.Exp, scale=0.5)
                nc.vector.tensor_scalar_mul(qsB[h][RSP:RSP + 1, n0:n0 + NCH],
                                            qex[:], -1.0)
        p_qsc.release()
        p_pq.release()
        p_psA.release()
        p_xs.release()
        p_wA.release()
        p_wKV.release()

        # --- Phase B: kv_b projection + k/v time rows + V' assembly ----------
        big2 = tc.alloc_tile_pool(name="big2", bufs=1)
        p_wB = tc.alloc_tile_pool(name="p_wB", bufs=1)
        p_psB = tc.alloc_tile_pool(name="p_psB", bufs=3, space="PSUM")
        p_pkv = tc.alloc_tile_pool(name="p_pkv", bufs=2, space="PSUM")
        p_bsc = tc.alloc_tile_pool(name="p_bsc", bufs=1)
        wb_k = []
        for k in range(4):
            w = p_wB.tile([P, WB_COLS], BF16, name=f"wbk_{k}", tag=f"wbk_{k}")
            nc.sync.dma_start(out=w[:], in_=wkvb_d[k * P:(k + 1) * P, :])
            wb_k.append(w)
        wb_t = p_wB.tile([1, WB_COLS], BF16, name="wb_t", tag="wb_t")
        nc.sync.dma_start(out=wb_t[:], in_=wkvb_d[KV_RANK:KV_RANK + 1, :])

        ksA = [big2.tile([P, S], BF16, name=f"ksA_{h}", tag=f"ksA_{h}") for h in range(HPC)]
        vts = [big2.tile([P, S], BF16, name=f"vts_{h}", tag=f"vts_{h}") for h in range(HPC)]

        def kvb_mms(ps, col0, msize, n0):
            for k in range(4):
                nc.tensor.matmul(ps[:msize, :], wb_k[k][:, col0:col0 + msize],
                                 kv[k][:, n0:n0 + NCH], start=(k == 0), stop=False)
            nc.tensor.matmul(ps[:msize, :], wb_t[:, col0:col0 + msize],
                             t_row_bf[:, n0:n0 + NCH], start=False, stop=True)

        for n in range(NA):
            n0 = n * NCH
            kpsq = p_bsc.tile([RSP, NCH], BF16, name="kpsq", tag="kpsq", bufs=2)
            nc.scalar.square(kpsq[:], kpe[:, n0:n0 + NCH])
            bsqs, vsqs = [], []
            for h in range(HPC):
                c0 = h * (NOPE + VSP)
                # k_nope
                ps = p_psB.tile([P, NCH], F32, name="psb", tag="psb", bufs=3)
                kvb_mms(ps, c0, NOPE, n0)
                nc.vector.tensor_copy(ksA[h][:, n0:n0 + NCH], ps[:])
                bsq = p_bsc.tile([P, NCH], BF16, name="bsq", tag="bsq", bufs=2)
                nc.scalar.square(bsq[:], ps[:])
                bsqs.append(bsq)
                # v (127 space rows; time goes in row 127 of vts)
                ps = p_psB.tile([P, NCH], F32, name="psb", tag="psb", bufs=3)
                kvb_mms(ps, c0 + NOPE, VSP, n0)
                nc.vector.tensor_copy(vts[h][0:VSP, n0:n0 + NCH], ps[:VSP, :])
                vsq = p_bsc.tile([VSP, NCH], BF16, name="vsq", tag="vsq", bufs=2)
                nc.scalar.square(vsq[:], ps[:VSP, :])
                vsqs.append(vsq)
            # k/v time rows (emitted after the projections so squares are ready)
            for h in range(HPC):
                pk = p_pkv.tile([1, NCH], F32, name="pk", tag="pk", bufs=2)
                nc.tensor.matmul(pk[:], ones_col[:], bsqs[h][:], start=True, stop=False)
                nc.tensor.matmul(pk[:], ones_col[0:RSP, :], kpsq[:],
                                 start=False, stop=True)
                kln = p_bsc.tile([1, NCH], F32, name="kln", tag="kln", bufs=2)
                nc.scalar.activation(kln[:], pk[:], AF.Ln, bias=1.0)
                nc.scalar.activation(ksB[h][RSP:RSP + 1, n0:n0 + NCH], kln[:],
                                     AF.Exp, scale=0.5)
                pv = p_pkv.tile([1, NCH], F32, name="pk", tag="pk", bufs=2)
                nc.tensor.matmul(pv[:], ones_col[0:VSP, :], vsqs[h][:],
                                 start=True, stop=True)
                vln = p_bsc.tile([1, NCH], F32, name="vln", tag="vln", bufs=2)
                nc.scalar.activation(vln[:], pv[:], AF.Ln, bias=1.0)
                # engines can't write a region based at partition 127; go via
                # a scratch row + SBUF->SBUF DMA
                vtr = p_bsc.tile([1, NCH], BF16, name="vtr", tag="vtr", bufs=2)
                nc.scalar.activation(vtr[:], vln[:], AF.Exp, scale=0.5)
                nc.sync.dma_start(out=vts[h][VSP:VSP + 1, n0:n0 + NCH],
                                  in_=vtr[:])
                # V' tiles for this chunk via DMA xbar transpose (off the PE)
                for j in range(n * 4, n * 4 + 4):
                    nc.sync.dma_start(out=Vp[h][:, j, :],
                                      in_=vts[h][:, j * P:(j + 1) * P],
                                      transpose=True)
        p_bsc.release()
        p_pkv.release()
        p_psB.release()
        p_wB.release()

        # ---------------- Phase C: attention ---------------------------------
        # scoresT layout [k, q]: lhsT and rhs are both [d, s] slices; exp(PSUM)
        # lands in SBUF already transposed for the AV matmul. The j-loop is
        # software-pipelined two deep so the PE never waits on the exp.
        GQ = NCH // P
        NG = S // NCH
        p_ex = tc.alloc_tile_pool(name="p_ex", bufs=4)
        p_cw = tc.alloc_tile_pool(name="p_cw", bufs=2)
        p_wO = tc.alloc_tile_pool(name="p_wO", bufs=1)
        p_osb = tc.alloc_tile_pool(name="p_osb", bufs=4)
        p_ave = tc.alloc_tile_pool(name="p_ave", bufs=1, space="PSUM")
        p_scp = tc.alloc_tile_pool(name="p_scp", bufs=3, space="PSUM")
        p_ptc = tc.alloc_tile_pool(name="p_ptc", bufs=1, space="PSUM")
        p_psD = tc.alloc_tile_pool(name="p_psD", bufs=3, space="PSUM")

        wo_sb = []
        for h in range(HPC):
            w = p_wO.tile([P, OUT_COLS], BF16, name=f"wo_{h}", tag=f"wo_{h}")
            nc.sync.dma_start(out=w[:], in_=wo_d[h * P:(h + 1) * P, :])
            wo_sb.append(w)

        junk = p_cw.tile([P, GQ, P], F32, name="junk", tag="junk", bufs=1)

        def sc_mms(g, h, j):
            lo = max(0, j - g * GQ) if causal else 0
            ncols = (GQ - lo) * P
            c0 = g * NCH + lo * P
            sc = p_scp.tile([P, NCH], F32, name="sc", tag="sc", bufs=3)
            nc.tensor.matmul(sc[:, :ncols], ksA[h][:, j * P:(j + 1) * P],
                             qsA[h][:, c0:c0 + ncols], start=True, stop=False)
            nc.tensor.matmul(sc[:, :ncols], ksB[h][:, j * P:(j + 1) * P],
                             qsB[h][:, c0:c0 + ncols], start=False, stop=True)
            return sc, lo, ncols

        def exp_av(g, h, j, ave, sc, lo, ncols, jmax):
            ex = p_ex.tile([P, NCH], BF16, name="ex", tag="ex", bufs=4)
            nc.scalar.activation(ex[:, :ncols], sc[:, :ncols], AF.Exp,
                                 scale=exp_scale)
            if causal and j >= g * GQ:
                nc.vector.tensor_mul(ex[:, 0:P], ex[:, 0:P], diagmask[:])
            for t in range(lo, GQ):
                qt_idx = g * GQ + t
                # start=True clears the WHOLE bank, so only the first
                # matmul of the packed accumulator issues it; the other
                # lanes overwrite-on-cleared-has_written at j == 0.
                nc.tensor.matmul(ave[:, t, :], ex[:, (t - lo) * P:(t - lo + 1) * P],
                                 Vp[h][:, j, :], start=(j == 0 and t == 0),
                                 stop=(j == (qt_idx if causal else jmax - 1)),
                                 skip_group_check=True)

        def epilogue(g, h, ave, cen_h):
            # rs = 1/sqrt(|inner|) per q row; |inner| = 2*t^2 - sum(ave^2)
            rsum = p_cw.tile([P, GQ], F32, name="rsum", tag="rsum", bufs=2)
            tsq = p_cw.tile([P, GQ], F32, name="tsq", tag="tsq", bufs=2)
            for t in range(GQ):
                nc.scalar.activation(junk[:, t, :], ave[:, t, :], AF.Square,
                                     accum_out=rsum[:, t:t + 1])
                nc.scalar.square(tsq[:, t:t + 1], ave[:, t, VSP:VSP + 1])
            innr = p_cw.tile([P, GQ], F32, name="innr", tag="innr", bufs=2)
            nc.vector.scalar_tensor_tensor(innr[:], tsq[:], 2.0, rsum[:],
                                           op0=ALU.mult, op1=ALU.subtract)
            lnr = p_cw.tile([P, GQ], F32, name="lnr", tag="lnr", bufs=2)
            nc.scalar.activation(lnr[:], innr[:], AF.Ln)
            rsv = p_cw.tile([P, GQ], F32, name="rsv", tag="rsv", bufs=2)
            nc.scalar.activation(rsv[:], lnr[:], AF.Exp, scale=-0.5)
            for t in range(GQ):
                nc.scalar.activation(cen_h[t][:], ave[:, t, :], AF.Copy,
                                     scale=rsv[:, t:t + 1])

        def wo_block(g, cen_g):
            for t in range(GQ):
                cts = []
                for h in range(HPC):
                    tpc = p_ptc.tile([P, P], BF16, name="tpc", tag="tpc", bufs=1)
                    nc.tensor.transpose(tpc[:], cen_g[h][t][:], identity[:])
                    ct = p_cw.tile([P, P], BF16, name="cenT", tag=f"cenT_{h}",
                                   bufs=2)
                    nc.vector.tensor_copy(ct[:], tpc[:])
                    cts.append(ct)
                m = g * GQ + t
                for n in range(4):
                    n0 = n * NCH
                    nn = min(NCH, OUT_COLS - n0)
                    ps = p_psD.tile([P, NCH], F32, name="psd", tag="psd", bufs=2)
                    nc.tensor.matmul(ps[:, :nn], cts[0][:],
                                     wo_sb[0][:, n0:n0 + nn], start=True, stop=False)
                    nc.tensor.matmul(ps[:, :nn], cts[1][:],
                                     wo_sb[1][:, n0:n0 + nn], start=False, stop=True)
                    ot = p_osb.tile([P, NCH], BF16, name="ot", tag="ot", bufs=6)
                    if n % 2 == 0:
                        nc.scalar.copy(ot[:, :nn], ps[:, :nn])
                    else:
                        nc.vector.tensor_copy(ot[:, :nn], ps[:, :nn])
                    nc.sync.dma_start(out=out_d[m * P:(m + 1) * P, n0:n0 + nn],
                                      in_=ot[:, :nn])

        prev_cen = None
        for g in range(NG):
            # cen tiles are double-buffered: group g's wo runs during group g+1
            cen_g = [[p_cw.tile([P, P], BF16, name=f"cenb_{h}_{t}",
                                tag=f"cenb_{h}_{t}", bufs=2)
                      for t in range(GQ)] for h in range(HPC)]
            for h in range(HPC):
                ave = p_ave.tile([P, GQ, P], F32, name="ave", tag="ave", bufs=2)
                jmax = (g * GQ + GQ) if causal else NQT
                pend = []
                for j in range(jmax):
                    pend.append((j,) + sc_mms(g, h, j))
                    if len(pend) > 2:
                        pj, psc, plo, pnc = pend.pop(0)
                        exp_av(g, h, pj, ave, psc, plo, pnc, jmax)
                for (pj, psc, plo, pnc) in pend:
                    exp_av(g, h, pj, ave, psc, plo, pnc, jmax)
                epilogue(g, h, ave, cen_g[h])
            if prev_cen is not None:
                wo_block(g - 1, prev_cen)
            prev_cen = cen_g
        wo_block(NG - 1, prev_cen)

        p_psD.release()
        p_ptc.release()
        p_scp.release()
        p_ave.release()
        p_osb.release()
        p_wO.release()
        p_cw.release()
        p_ex.release()

        big2.release()
        big.release()
        const.release()

    nc.compile()
    return nc


_CACHE = {}


def _get_program(exp_scale: float, causal: bool):
    key = (round(float(exp_scale), 12), causal)
    if key not in _CACHE:
        _CACHE[key] = _build_program(float(exp_scale), causal)
    return _CACHE[key]


def _rope_perm():
    """Even rope dims first, then odd (host-side column permutation)."""
    return np.concatenate([np.arange(0, RSP, 2), np.arange(1, RSP, 2)])


def kernel(x, start_pos, freqs_cos, freqs_sin, mask, wq_w, wkv_a_w, kv_norm_w,
           wkv_b_w, wo_w, softmax_scale, bias_p, _want_trace=False):
    x2 = np.ascontiguousarray(np.asarray(x, np.float32).reshape(S, DIM))
    xT = np.ascontiguousarray(x2.T)
    wq_w = np.asarray(wq_w, np.float32)
    wkv_a_w = np.asarray(wkv_a_w, np.float32)
    kv_norm_w = np.asarray(kv_norm_w, np.float32)
    wkv_b_w = np.asarray(wkv_b_w, np.float32)
    wo_w = np.asarray(wo_w, np.float32)
    cosT = np.asarray(freqs_cos, np.float32).T
    sinT = np.asarray(freqs_sin, np.float32).T
    cosT = np.ascontiguousarray(
        np.concatenate([cosT, cosT], axis=0).astype(ml_dtypes.bfloat16))
    sinT = np.ascontiguousarray(
        np.concatenate([sinT, sinT], axis=0).astype(ml_dtypes.bfloat16))

    mask = np.asarray(mask)
    causal = bool(np.array_equal(mask, np.triu(np.ones((S, S), bool), k=1)))
    if not causal:
        assert not mask.any(), "only causal or empty masks are supported"

    smax = float(np.asarray(softmax_scale).reshape(-1)[0])
    exp_scale = 2.0 / smax

    rp = _rope_perm()
    # wq per core-pair layout: [nope_h0 | nope_h1 | rope_h0(ev,od) | rope_h1(ev,od)]
    wq_r = wq_w.reshape(DIM, 16, QH)
    wq_nope = wq_r[:, :, :NOPE]                       # (DIM, 16, 128)
    wq_rope = wq_r[:, :, NOPE:][:, :, rp]             # (DIM, 16, 64) permuted
    wq_cores = []
    for c in range(N_CORES):
        h0, h1 = 2 * c, 2 * c + 1
        wq_cores.append(np.concatenate(
            [wq_nope[:, h0], wq_nope[:, h1], wq_rope[:, h0], wq_rope[:, h1]],
            axis=1))
    # wkva: [kv | rope-even | rope-odd]
    wkva_p = wkv_a_w.copy()
    wkva_p[:, KV_RANK:] = wkva_p[:, KV_RANK:][:, rp]
    # wkvb: kvn rows first, time row last
    wkvb_p = np.ascontiguousarray(np.concatenate([wkv_b_w[1:], wkv_b_w[:1]], axis=0))
    wnormT = np.ascontiguousarray(kv_norm_w.reshape(4, P).T)
    # wo rows per head: [v space (1..127), time (0)]
    wo_p = wo_w.reshape(16, P, OUT_COLS)
    wo_p = np.concatenate([wo_p[:, 1:, :], wo_p[:, 0:1, :]], axis=1)
    wo_p = wo_p.reshape(16 * P, OUT_COLS)

    nc = _get_program(exp_scale, causal)

    xT_bf = np.ascontiguousarray(xT.astype(ml_dtypes.bfloat16))
    wkva_bf = np.ascontiguousarray(wkva_p.astype(ml_dtypes.bfloat16))

    in_maps = []
    for c in range(N_CORES):
        in_maps.append({
            "xT": xT_bf,
            "wq": np.ascontiguousarray(wq_cores[c].astype(ml_dtypes.bfloat16)),
            "wkva": wkva_bf,
            "wnormT": wnormT,
            "wkvb": np.ascontiguousarray(
                wkvb_p[:, c * WB_COLS:(c + 1) * WB_COLS].astype(ml_dtypes.bfloat16)),
            "wo": np.ascontiguousarray(
                wo_p[c * WO_ROWS:(c + 1) * WO_ROWS, :].astype(ml_dtypes.bfloat16)),
            "cosT": cosT,
            "sinT": sinT,
            "xsl": np.ascontiguousarray(xT_bf[:, c * SL:(c + 1) * SL]),
            "cossl": np.ascontiguousarray(cosT[:, c * SL:(c + 1) * SL]),
            "sinsl": np.ascontiguousarray(sinT[:, c * SL:(c + 1) * SL]),
        })

    res = run_bass_kernel_spmd(nc, in_maps, core_ids=list(range(N_CORES)),
                               trace=_want_trace)
    kernel.last_result = res

    total = res.results[0]["out"].astype(np.float32)
    for c in range(1, N_CORES):
        total = total + res.results[c]["out"].astype(np.float32)
    t = np.sqrt(np.sum(total * total, axis=-1, keepdims=True) + 1.0)
    out = np.concatenate([t, total], axis=-1)
    return out.reshape(1, S, DIM).astype(np.float32)
